# revision 1
# baseline (speedup 1.0000x reference)
"""Trainium2 Bass kernel for quantized linear: y_q = sat_int8(round((x_q @ w_q^T) * scale)).

Strategy (8 NeuronCores, tensor-parallel over out_features):
  - Each core gets rows [c*512, (c+1)*512) of weight_q and the full x_q;
    it computes its [8192, 512] slice of the output with a fused epilogue.
  - int8 x int8 matmul is not supported by the PE, but every int8 value is
    exact in bf16, products (<= 2^14) are exact, and all fp32 PSUM partial
    sums stay far below 2^24, so a bf16 GEMM reproduces the int32-exact
    accumulation bit-for-bit.
  - Epilogue per [128, 512] tile: ACT computes acc*scale + 1.5*2^23 (the
    fp32 magic-number round-to-nearest-even), DVE clamps to the int8 range
    (still offset by the magic constant), ACT subtracts the constant and
    casts to int8. This matches jnp.round + clip + astype(int8) exactly.
"""

import os

import ml_dtypes
import numpy as np

import concourse.mybir as mybir
import concourse.tile as tile
from concourse import bacc
from concourse.bass_utils import run_bass_kernel_spmd

M_TOKENS = 8192
IN_FEATURES = 4096
OUT_FEATURES = 4096
WEIGHT_SCALE = 0.1
OUT_SCALE = 0.1

N_CORES = 8
N_PER = OUT_FEATURES // N_CORES  # 512 output features per core
P = 128
KT = IN_FEATURES // P            # 32 k-tiles
MB = 16                          # m-blocks (DMA granularity)
MB_TOK = M_TOKENS // MB          # 512 tokens per m-block
MT_PER_MB = MB_TOK // P          # 4 m-tiles per block
NF = N_PER                       # matmul free dim = 512 (one PSUM bank fp32)
MAGIC = 12582912.0               # 1.5 * 2**23
BF16 = ml_dtypes.bfloat16


def _build(scale: float):
    nc = bacc.Bacc("TRN2", target_bir_lowering=False, debug=False,
                   enable_asserts=True, num_devices=N_CORES)

    # Register fp32 consts so ACT Identity can take them as bias APs.
    for v in (MAGIC, -MAGIC):
        t = nc.alloc_sbuf_tensor(f"const-f32-{v}", [P, 1], mybir.dt.float32)
        nc.gpsimd.memset(t.ap(), v)
        nc.const_aps.aps[(mybir.dt.float32, v)] = t.ap()
    nc.all_engine_barrier()

    # x_t[mb, p, kt, mi] = x[mb*512 + mi, kt*128 + p]  (bf16)
    x_t = nc.dram_tensor("x_t", [MB, P, KT, MB_TOK], mybir.dt.bfloat16,
                         kind="ExternalInput")
    # w_t[p, kt, n] = w[core*512 + n, kt*128 + p]  (bf16, per-core slice)
    w_t = nc.dram_tensor("w_t", [P, KT, N_PER], mybir.dt.bfloat16,
                         kind="ExternalInput")
    y = nc.dram_tensor("y", [M_TOKENS, N_PER], mybir.dt.int8,
                       kind="ExternalOutput")

    with tile.TileContext(nc) as tc:
        with tc.tile_pool(name="wpool", bufs=1) as wpool, \
             tc.tile_pool(name="xpool", bufs=3) as xpool, \
             tc.tile_pool(name="pp", bufs=4, space="PSUM") as pp, \
             tc.tile_pool(name="epi", bufs=4) as epi, \
             tc.tile_pool(name="op", bufs=4) as op:
            wsb = wpool.tile([P, KT, N_PER], mybir.dt.bfloat16)
            nc.sync.dma_start(wsb[:], w_t[:])
            for mb in range(MB):
                xsb = xpool.tile([P, KT, MB_TOK], mybir.dt.bfloat16)
                nc.sync.dma_start(xsb[:], x_t[mb])
                for mt in range(MT_PER_MB):
                    ps = pp.tile([P, NF], mybir.dt.float32)
                    for kt in range(KT):
                        nc.tensor.matmul(
                            ps[:], xsb[:, kt, mt * P:(mt + 1) * P], wsb[:, kt, :],
                            start=(kt == 0), stop=(kt == KT - 1))
                    f = epi.tile([P, NF], mybir.dt.float32, tag="f")
                    nc.scalar.activation(
                        f[:], ps[:], mybir.ActivationFunctionType.Identity,
                        bias=MAGIC, scale=scale)
                    g = epi.tile([P, NF], mybir.dt.float32, tag="g")
                    nc.vector.tensor_scalar(
                        g[:], f[:], MAGIC + 127.0, MAGIC - 128.0,
                        mybir.AluOpType.min, mybir.AluOpType.max)
                    o = op.tile([P, NF], mybir.dt.int8)
                    nc.scalar.activation(
                        o[:], g[:], mybir.ActivationFunctionType.Identity,
                        bias=-MAGIC, scale=1.0)
                    row = (mb * MT_PER_MB + mt) * P
                    nc.sync.dma_start(y[row:row + P, :], o[:])
    nc.compile()
    return nc


def _prep_inputs(x_q: np.ndarray, weight_q: np.ndarray):
    # int32 carriers -> int8 -> bf16 (both conversions exact for [-128, 127])
    x8 = x_q.astype(np.int8, copy=False)
    xb = x8.astype(BF16)
    # [M, K] -> [mb, p, kt, mi]
    x_t = np.ascontiguousarray(
        xb.reshape(MB, MB_TOK, KT, P).transpose(0, 3, 2, 1))
    w8 = weight_q.astype(np.int8, copy=False)
    wb = w8.astype(BF16)
    w_ts = []
    for c in range(N_CORES):
        wc = wb[c * N_PER:(c + 1) * N_PER, :]       # [n, k]
        w_ts.append(np.ascontiguousarray(
            wc.reshape(N_PER, KT, P).transpose(2, 1, 0)))  # [p, kt, n]
    return x_t, w_ts


def _run(x_q, weight_q, scale_x, trace=False):
    x_q = np.asarray(x_q)
    weight_q = np.asarray(weight_q)
    # Match the reference's fp32 arithmetic: scale_x * f32(0.1) / f32(0.1)
    scale = float(np.float32(scale_x) * np.float32(WEIGHT_SCALE)
                  / np.float32(OUT_SCALE))
    nc = _build(scale)
    x_t, w_ts = _prep_inputs(x_q, weight_q)
    in_maps = [{"x_t": x_t, "w_t": w_ts[c]} for c in range(N_CORES)]
    res = run_bass_kernel_spmd(nc, in_maps, core_ids=list(range(N_CORES)),
                               trace=trace)
    y_full = np.concatenate([res.results[c]["y"] for c in range(N_CORES)],
                            axis=1)
    return (y_full, np.float32(OUT_SCALE)), res


def kernel(x_q, weight_q, scale_x):
    trace = bool(os.environ.get("KERNEL_TRACE"))
    out, _ = _run(x_q, weight_q, scale_x, trace=trace)
    return out


# revision 3
# speedup vs baseline: 1.0376x; 1.0376x over previous
"""Trainium2 Bass kernel for quantized linear: y_q = sat_int8(round((x_q @ w_q^T) * scale)).

Strategy (8 NeuronCores, tensor-parallel over out_features):
  - Each core gets rows [c*512, (c+1)*512) of weight_q and the full x_q;
    it computes its [8192, 512] slice of the output with a fused epilogue.
  - int8 x int8 matmul is not supported by the PE, but every int8 value is
    exact in bf16, products (<= 2^14) are exact, and all fp32 PSUM partial
    sums stay far below 2^24, so a bf16 GEMM reproduces the int32-exact
    accumulation bit-for-bit.
  - Epilogue per [128, 512] tile: ACT computes acc*scale + 1.5*2^23 (the
    fp32 magic-number round-to-nearest-even), DVE clamps to the int8 range
    (still offset by the magic constant), ACT subtracts the constant and
    casts to int8. This matches jnp.round + clip + astype(int8) exactly.
  - Weights and the x m-blocks are loaded in k-chunks on two HWDGE rings
    (sync + scalar) so the first matmuls start as soon as the first chunks
    land; m-block 0 iterates chunk-outer/m-tile-inner to consume chunks at
    arrival rate.
"""

import os

import ml_dtypes
import numpy as np

import concourse.mybir as mybir
import concourse.tile as tile
from concourse import bacc
from concourse.bass_utils import run_bass_kernel_spmd

M_TOKENS = 8192
IN_FEATURES = 4096
OUT_FEATURES = 4096
WEIGHT_SCALE = 0.1
OUT_SCALE = 0.1

N_CORES = 8
N_PER = OUT_FEATURES // N_CORES  # 512 output features per core
P = 128
KT = IN_FEATURES // P            # 32 k-tiles
CH = 4                           # k-tiles per DMA chunk
NCH = KT // CH                   # 8 chunks
MB = 16                          # m-blocks (DMA granularity)
MB_TOK = M_TOKENS // MB          # 512 tokens per m-block
MT_PER_MB = MB_TOK // P          # 4 m-tiles per block
NF = N_PER                       # matmul free dim = 512 (one PSUM bank fp32)
MAGIC = 12582912.0               # 1.5 * 2**23
BF16 = ml_dtypes.bfloat16
F32 = mybir.dt.float32
ACT_COPY = mybir.ActivationFunctionType.Copy


def _build(scale: float):
    nc = bacc.Bacc("TRN2", target_bir_lowering=False, debug=False,
                   enable_asserts=True, num_devices=N_CORES)

    # x_t[mb, p, kt, mi] = x[mb*512 + mi, kt*128 + p]  (bf16)
    x_t = nc.dram_tensor("x_t", [MB, P, KT, MB_TOK], mybir.dt.bfloat16,
                         kind="ExternalInput")
    # w_t[p, kt, n] = w[core*512 + n, kt*128 + p]  (bf16, per-core slice)
    w_t = nc.dram_tensor("w_t", [P, KT, N_PER], mybir.dt.bfloat16,
                         kind="ExternalInput")
    y = nc.dram_tensor("y", [M_TOKENS, N_PER], mybir.dt.int8,
                       kind="ExternalOutput")

    with tile.TileContext(nc) as tc:
        with tc.tile_pool(name="wpool", bufs=NCH) as wpool, \
             tc.tile_pool(name="xpool", bufs=3 * NCH) as xpool, \
             tc.tile_pool(name="pp", bufs=8, space="PSUM") as pp, \
             tc.tile_pool(name="epi", bufs=4) as epi, \
             tc.tile_pool(name="op", bufs=4) as op:

            def load_x_chunks(mb):
                ts = []
                for c in range(NCH):
                    xt = xpool.tile([P, CH, MB_TOK], mybir.dt.bfloat16, tag="x")
                    nc.scalar.dma_start(xt[:], x_t[mb, :, c * CH:(c + 1) * CH, :])
                    ts.append(xt)
                return ts

            def epilogue(ps, row):
                f = epi.tile([P, NF], F32, tag="f")
                nc.scalar.activation(f[:], ps[:], ACT_COPY,
                                     bias=MAGIC, scale=scale)
                g = epi.tile([P, NF], F32, tag="g")
                nc.vector.tensor_scalar(g[:], f[:], MAGIC + 127.0, MAGIC - 128.0,
                                        mybir.AluOpType.min, mybir.AluOpType.max)
                o = op.tile([P, NF], mybir.dt.int8, tag="o")
                nc.scalar.activation(o[:], g[:], ACT_COPY, bias=-MAGIC, scale=1.0)
                nc.sync.dma_start(y[row:row + P, :], o[:])

            # Interleave w / x(mb=0) chunk loads on the two HWDGE rings so
            # the first matmul's operands land first.
            wt = []
            x0t = []
            for c in range(NCH):
                w1 = wpool.tile([P, CH, N_PER], mybir.dt.bfloat16, tag="w")
                nc.sync.dma_start(w1[:], w_t[:, c * CH:(c + 1) * CH, :])
                wt.append(w1)
                x1 = xpool.tile([P, CH, MB_TOK], mybir.dt.bfloat16, tag="x")
                nc.scalar.dma_start(x1[:], x_t[0, :, c * CH:(c + 1) * CH, :])
                x0t.append(x1)

            # m-block 0: chunk-outer / m-tile-inner, consumes chunks as they
            # arrive. Uses 4 PSUM banks simultaneously.
            ps0 = [pp.tile([P, NF], F32, tag="ps", name=f"ps0_{i}")
                   for i in range(MT_PER_MB)]
            for c in range(NCH):
                for mt in range(MT_PER_MB):
                    for k in range(CH):
                        nc.tensor.matmul(
                            ps0[mt][:], x0t[c][:, k, mt * P:(mt + 1) * P],
                            wt[c][:, k, :],
                            start=(c == 0 and k == 0),
                            stop=(c == NCH - 1 and k == CH - 1))
            xt_next = load_x_chunks(1)
            for mt in range(MT_PER_MB):
                epilogue(ps0[mt], mt * P)

            # m-blocks 1..15: standard m-tile-outer / k-inner with epilogue
            # per m-tile; prefetch next block's chunks.
            for mb in range(1, MB):
                xt = xt_next
                if mb + 1 < MB:
                    xt_next = load_x_chunks(mb + 1)
                for mt in range(MT_PER_MB):
                    ps = pp.tile([P, NF], F32, tag="ps")
                    for kt in range(KT):
                        c, k = divmod(kt, CH)
                        nc.tensor.matmul(
                            ps[:], xt[c][:, k, mt * P:(mt + 1) * P],
                            wt[c][:, k, :],
                            start=(kt == 0), stop=(kt == KT - 1))
                    epilogue(ps, (mb * MT_PER_MB + mt) * P)
    nc.compile()
    return nc


def _prep_inputs(x_q: np.ndarray, weight_q: np.ndarray):
    # int32 carriers -> int8 -> bf16 (both conversions exact for [-128, 127])
    x8 = x_q.astype(np.int8, copy=False)
    xb = x8.astype(BF16)
    # [M, K] -> [mb, p, kt, mi]
    x_t = np.ascontiguousarray(
        xb.reshape(MB, MB_TOK, KT, P).transpose(0, 3, 2, 1))
    w8 = weight_q.astype(np.int8, copy=False)
    wb = w8.astype(BF16)
    w_ts = []
    for c in range(N_CORES):
        wc = wb[c * N_PER:(c + 1) * N_PER, :]       # [n, k]
        w_ts.append(np.ascontiguousarray(
            wc.reshape(N_PER, KT, P).transpose(2, 1, 0)))  # [p, kt, n]
    return x_t, w_ts


def _run(x_q, weight_q, scale_x, trace=False):
    x_q = np.asarray(x_q)
    weight_q = np.asarray(weight_q)
    # Match the reference's fp32 arithmetic: scale_x * f32(0.1) / f32(0.1)
    scale = float(np.float32(scale_x) * np.float32(WEIGHT_SCALE)
                  / np.float32(OUT_SCALE))
    nc = _build(scale)
    x_t, w_ts = _prep_inputs(x_q, weight_q)
    in_maps = [{"x_t": x_t, "w_t": w_ts[c]} for c in range(N_CORES)]
    res = run_bass_kernel_spmd(nc, in_maps, core_ids=list(range(N_CORES)),
                               trace=trace)
    y_full = np.concatenate([res.results[c]["y"] for c in range(N_CORES)],
                            axis=1)
    return (y_full, np.float32(OUT_SCALE)), res


def kernel(x_q, weight_q, scale_x):
    trace = bool(os.environ.get("KERNEL_TRACE"))
    out, _ = _run(x_q, weight_q, scale_x, trace=trace)
    return out
